# revision 1
# baseline (speedup 1.0000x reference)
"""Trainium2 Bass kernel for nn_Amplified_PatternMixer.

Computation:
  out[b, h, m1, m2] = mixed_pattern[h, m1, m2] + alpha[h] * nrm[b, m2]
where
  nrm[b, m] = || mean_{hw}(x[b*57+m, :, h, w]) ||_2   over channels
  mixed_pattern = tiny 57x57 graph-normalized pattern (from 5x7x7 params).

The memory-bound part (streaming x: [1824, 256, 14, 14] f32, ~366 MB) runs
on 8 NeuronCores, data-parallel over rows (228 rows/core).  Each core:
  - DMAs its row shard in channel-chunks to SBUF (rows on partitions)
  - reduce_sum over the 196-elem HW dim per channel -> channel sums
  - square + reduce over channels, sqrt, scale by 1/196 -> per-row norm
The tiny pattern-mixer math (57x57, a few thousand flops) runs on host.
"""

import os

import numpy as np

import concourse.bacc as bacc
import concourse.bass as bass
import concourse.mybir as mybir
import concourse.tile as tile
from concourse.bass_utils import run_bass_kernel_spmd

# Problem constants (hardcoded; kernel.py must be self-contained).
NUM_BASIC = 5
NUM_MIXED = 4
NUM_FRAME = 8
NUM_NODES = 7
NUM_SAMPLES = 8
M = 1 + NUM_NODES * NUM_FRAME  # 57

N_CORES = 8
B = 32
C = 256
HW = 196  # 14*14
ROWS_TOTAL = B * M          # 1824
ROWS_PER_CORE = ROWS_TOTAL // N_CORES  # 228
CW = C * HW                 # 50176 floats per row
G = 64                      # channels per SBUF chunk
N_CHUNKS = C // G
ROW_GROUPS = [(0, 128), (128, ROWS_PER_CORE - 128)]

LAST_RESULT = None
_NC_CACHE = None


def _build_nc(chunks=None, bufs=3, dma_engines=("gpsimd",)):
    """Build the per-core norm kernel.

    chunks: list of channel-chunk sizes summing to C (default [G]*N_CHUNKS)
    bufs: xt tile pool depth
    dma_engines: cycle of engine names for chunk-load DMAs
    """
    if chunks is None:
        chunks = [G] * N_CHUNKS
    assert sum(chunks) == C
    nc = bacc.Bacc(None)
    x = nc.declare_dram_parameter(
        "x", [ROWS_PER_CORE, CW], mybir.dt.float32, isOutput=False
    )
    out = nc.declare_dram_parameter(
        "out", [ROWS_PER_CORE, 1], mybir.dt.float32, isOutput=True
    )
    max_g = max(chunks)
    with tile.TileContext(nc) as tc:
        with (
            tc.tile_pool(name="xt_pool", bufs=bufs) as xp,
            tc.tile_pool(name="acc_pool", bufs=2) as accp,
            tc.tile_pool(name="res_pool", bufs=2) as resp,
        ):
            di = 0
            for r0, P in ROW_GROUPS:
                cs = accp.tile([128, C], mybir.dt.float32, tag="cs")
                c0 = 0
                for g in chunks:
                    xt = xp.tile([128, max_g * HW], mybir.dt.float32, tag="xt")
                    eng = getattr(nc, dma_engines[di % len(dma_engines)])
                    di += 1
                    eng.dma_start(
                        out=xt[:P, : g * HW],
                        in_=x[r0 : r0 + P, c0 * HW : (c0 + g) * HW],
                    )
                    nc.vector.reduce_sum(
                        cs[:P, c0 : c0 + g],
                        xt[:P, : g * HW].rearrange("p (g w) -> p g w", w=HW),
                        axis=mybir.AxisListType.X,
                    )
                    c0 += g
                sq = accp.tile([128, C], mybir.dt.float32, tag="sq")
                nc.vector.tensor_mul(sq[:P], cs[:P], cs[:P])
                ss = resp.tile([128, 1], mybir.dt.float32, tag="ss")
                nc.vector.reduce_sum(ss[:P], sq[:P], axis=mybir.AxisListType.X)
                nrm = resp.tile([128, 1], mybir.dt.float32, tag="nrm")
                # nrm = sqrt(ss / 196^2) = sqrt(ss) / 196
                nc.scalar.activation(
                    nrm[:P],
                    ss[:P],
                    mybir.ActivationFunctionType.Sqrt,
                    scale=1.0 / float(HW * HW),
                )
                nc.gpsimd.dma_start(out=out[r0 : r0 + P, :], in_=nrm[:P])
    nc.finalize()
    return nc


def _build_nc_striped(
    chunks=(96, 96, 64),
    bufs=2,
    chunks_b=(64, 64, 64, 64),
    quarter_a=False,
    act_per_chunk=0,
    fold=False,
    scratch=16384,
    first_whole=False,
):
    """Engine-balanced variant.  Rows 0..127 -> partitions 0..127 (tile A).
    Rows 128..223 -> partition 4*bk+s for s in 0..2 (3-of-4 stripe); rows
    224..227 -> partitions 115/119/123/127 (tile B).  This equalizes the
    per-DMA-engine byte load (each engine serves a fixed set of partitions),
    raising the floor from 16 rows/engine worst case to 15."""
    chunks = list(chunks)
    chunks_b = list(chunks_b)
    assert sum(chunks) == C and sum(chunks_b) == C
    # striped-tile descriptors must not exceed 64KB (per-partition run would
    # split, adding a 4th AP dim the DMA balancer can't handle)
    assert all(g * HW * 4 <= 65536 for g in chunks_b)
    nc = bacc.Bacc(None, dynamic_dma_scratch_size=scratch)
    x = nc.declare_dram_parameter(
        "x", [ROWS_PER_CORE, CW], mybir.dt.float32, isOutput=False
    )
    out = nc.declare_dram_parameter(
        "out", [ROWS_PER_CORE, 1], mybir.dt.float32, isOutput=True
    )
    max_g = max(max(chunks), max(chunks_b))
    x_str = x[128:224].rearrange("(a b) c -> a b c", b=3)  # [32, 3, CW]
    out_str = out[128:224].rearrange("(a b) o -> a b o", b=3)  # [32, 3, 1]
    with tile.TileContext(nc) as tc:
        with (
            tc.tile_pool(name="xt_pool", bufs=bufs) as xp,
            tc.tile_pool(name="acc_pool", bufs=2) as accp,
            tc.tile_pool(name="res_pool", bufs=2) as resp,
            tc.tile_pool(name="trash_pool", bufs=2) as trp,
        ):
            for grp in range(2):
                # Group 1's unused stripe partitions (s=3, bk<28) reduce stale
                # tile-A data: finite garbage, never DMA'd out.  Tile A writes
                # all 128 partitions first, so nothing uninitialized is read.
                cs = accp.tile([128, C], mybir.dt.float32, tag="cs")
                c0 = 0
                for g in chunks if grp == 0 else chunks_b:
                    xt = xp.tile([128, max_g * HW], mybir.dt.float32, tag="xt")
                    if grp == 0:
                        if first_whole and c0 == 0:
                            # single 128-partition DMA for the very first
                            # chunk: one instruction fills all 16 engine
                            # rings at once (kills the startup stutter)
                            nc.gpsimd.dma_start(
                                out=xt[:, : g * HW],
                                in_=x[0:128, c0 * HW : (c0 + g) * HW],
                            )
                        elif quarter_a:
                            # stride-N stripes: splitting each chunk load into
                            # several strided partition-subset DMAs deepens the
                            # per-SDMA-engine descriptor queue (higher rate)
                            nsplit = 8 if quarter_a == 8 else 4
                            xta = xt.rearrange("(a b) f -> a b f", b=nsplit)
                            xa = x[0:128].rearrange("(a b) c -> a b c", b=nsplit)
                            for q in range(nsplit):
                                nc.gpsimd.dma_start(
                                    out=xta[:, q, : g * HW],
                                    in_=xa[:, q, c0 * HW : (c0 + g) * HW],
                                )
                        else:
                            nc.gpsimd.dma_start(
                                out=xt[:, : g * HW],
                                in_=x[0:128, c0 * HW : (c0 + g) * HW],
                            )
                    else:
                        # one DMA per stripe offset s: single partition dim
                        # (stride 4, count 32) — multi-dim partition APs
                        # mislower in the DMA descriptor generator
                        xt4 = xt.rearrange("(a b) f -> a b f", b=4)
                        for s in range(3):
                            nc.gpsimd.dma_start(
                                out=xt4[:, s, : g * HW],
                                in_=x_str[:, s, c0 * HW : (c0 + g) * HW],
                            )
                        nc.gpsimd.dma_start(
                            out=xt4[28:32, 3, : g * HW],
                            in_=x[224:228, c0 * HW : (c0 + g) * HW],
                        )
                    d = g - min(act_per_chunk, g)
                    if fold and d > 0:
                        # GpSimd folds hw 196 -> 98 (DVE work halves);
                        # POOL is otherwise only doing SWDGE descriptor gen
                        fw = HW // 2
                        ft = trp.tile(
                            [128, max_g * fw], mybir.dt.float32, tag="fold"
                        )
                        x3 = xt[:, : d * HW].rearrange("p (g w) -> p g w", w=HW)
                        nc.gpsimd.tensor_add(
                            ft[:, : d * fw].rearrange("p (g w) -> p g w", w=fw),
                            x3[:, :, 0:fw],
                            x3[:, :, fw : 2 * fw],
                        )
                        nc.vector.reduce_sum(
                            cs[:, c0 : c0 + d],
                            ft[:, : d * fw].rearrange("p (g w) -> p g w", w=fw),
                            axis=mybir.AxisListType.X,
                        )
                    elif d > 0:
                        nc.vector.reduce_sum(
                            cs[:, c0 : c0 + d],
                            xt[:, : d * HW].rearrange("p (g w) -> p g w", w=HW),
                            axis=mybir.AxisListType.X,
                        )
                    if d < g:
                        # ScalarE handles the tail channels: Copy-activation
                        # with accum_out gives the per-partition sum
                        trash = trp.tile([128, HW], mybir.dt.float32, tag="tr")
                        for ch in range(d, g):
                            nc.scalar.activation(
                                trash,
                                xt[:, ch * HW : (ch + 1) * HW],
                                mybir.ActivationFunctionType.Copy,
                                accum_out=cs[:, c0 + ch : c0 + ch + 1],
                            )
                    c0 += g
                sq = accp.tile([128, C], mybir.dt.float32, tag="sq")
                nc.vector.tensor_mul(sq, cs, cs)
                ss = resp.tile([128, 1], mybir.dt.float32, tag="ss")
                nc.vector.reduce_sum(ss, sq, axis=mybir.AxisListType.X)
                nrm = resp.tile([128, 1], mybir.dt.float32, tag="nrm")
                nc.scalar.activation(
                    nrm,
                    ss,
                    mybir.ActivationFunctionType.Sqrt,
                    scale=1.0 / float(HW * HW),
                )
                if grp == 0:
                    # sync-queue (HWDGE) for outputs: keeps the gpsimd load
                    # queue free so group 1's loads issue behind group 0's
                    nc.sync.dma_start(out=out[0:128, :], in_=nrm)
                else:
                    nrm4 = nrm.rearrange("(a b) o -> a b o", b=4)
                    for s in range(3):
                        eng = nc.sync if s % 2 == 0 else nc.scalar
                        eng.dma_start(out=out_str[:, s, :], in_=nrm4[:, s, :])
                    nc.scalar.dma_start(
                        out=out[224:228, :], in_=nrm4[28:32, 3, :]
                    )
    nc.finalize()
    return nc


def _get_nc():
    global _NC_CACHE
    if _NC_CACHE is None:
        # Best measured config: engine-balanced striped layout, 64-channel
        # chunks, 32-partition stride-4 DMAs, triple-buffered, small tail
        # chunks to shorten the final reduce lag.
        _NC_CACHE = _build_nc_striped(
            chunks=[48, 48, 48, 48, 48, 16],
            chunks_b=[48, 48, 48, 48, 32, 16, 8, 8],
            bufs=4,
            quarter_a=True,
            first_whole=True,
        )
    return _NC_CACHE


def _zero_mask():
    mask = np.ones((M, M), dtype=np.float64)
    for i in range(NUM_SAMPLES):
        r = (1 + i) * NUM_NODES
        for c in range(1, M):
            if c % NUM_NODES != 0 and (c - 1) // NUM_NODES != i:
                mask[r, c] = 0.0
    return mask


def _pattern_mixer_np(mat, sigma, lin_w, lin_b, mixed_mat):
    mat = np.asarray(mat, np.float64)            # [5, 7, 7]
    sigma = np.asarray(sigma, np.float64)        # [4, 5, 1]
    lin_w = np.asarray(lin_w, np.float64)        # [4, 5]
    lin_b = np.asarray(lin_b, np.float64)        # [4]
    mixed_mat = np.asarray(mixed_mat, np.float64)  # [4, 57, 57]

    T2 = 2 * NUM_FRAME - 1  # 15
    dist = np.abs(np.arange(T2, dtype=np.float64) - (NUM_FRAME - 1))
    te = (1.0 / (np.sqrt(2.0 * np.pi) * sigma)) * np.exp(
        -(dist**2) / (2.0 * sigma**2)
    )  # [4, 5, 15]
    ce = 1.0 / (1.0 + np.exp(-te))
    mixed = (
        np.einsum("hbt,bnm,hb->hntm", ce, mat, lin_w)
        + lin_b[:, None, None, None]
    )
    mixed = np.maximum(mixed, 0.0).reshape(NUM_MIXED, NUM_NODES, T2 * NUM_NODES)
    blocks = [
        mixed[
            :,
            :,
            NUM_NODES * (NUM_SAMPLES - 1 - i) : NUM_NODES * (2 * NUM_SAMPLES - 1 - i),
        ]
        for i in range(NUM_SAMPLES)
    ]
    add_block = np.concatenate(blocks, axis=1)  # [4, 56, 56]
    mm = mixed_mat.copy()
    mm[:, 1:, 1:] += add_block
    mm *= _zero_mask()[None]
    deg = np.maximum(mm.sum(axis=2), 1.0) ** -0.5  # [4, 57]
    return (deg[:, :, None] * mm * deg[:, None, :]).astype(np.float32)


def kernel(mat, x, sigma, lin_w, lin_b, mixed_mat, alpha):
    global LAST_RESULT
    x = np.ascontiguousarray(np.asarray(x, dtype=np.float32))
    xs = x.reshape(ROWS_TOTAL, CW)
    in_maps = [
        {"x": xs[i * ROWS_PER_CORE : (i + 1) * ROWS_PER_CORE]} for i in range(N_CORES)
    ]
    nc = _get_nc()
    res = run_bass_kernel_spmd(nc, in_maps, core_ids=list(range(N_CORES)))
    LAST_RESULT = res
    norms = np.concatenate([r["out"][:, 0] for r in res.results])  # [1824]
    nrm = norms.reshape(B, M)

    mp = _pattern_mixer_np(mat, sigma, lin_w, lin_b, mixed_mat)  # [4, 57, 57] f32
    alpha = np.asarray(alpha, np.float32).reshape(1, NUM_MIXED, 1, 1)
    out = mp[None] + alpha * nrm[:, None, None, :]
    return np.ascontiguousarray(out.astype(np.float32))



# revision 3
# speedup vs baseline: 3.3057x; 3.3057x over previous
"""Trainium2 Bass kernel for nn_Amplified_PatternMixer.

Computation:
  out[b, h, m1, m2] = mixed_pattern[h, m1, m2] + alpha[h] * nrm[b, m2]
where
  nrm[b, m] = || mean_{hw}(x[b*57+m, :, h, w]) ||_2   over channels
  mixed_pattern = tiny 57x57 graph-normalized pattern (from 5x7x7 params).

The memory-bound part (streaming x: [1824, 256, 14, 14]) runs on 8
NeuronCores, data-parallel over rows (228 rows/core).  The norm is highly
error-tolerant (harness gate: rel_err < 2e-2), so x is streamed as bf16
(measured end-to-end rel err ~3e-4), halving HBM traffic vs f32.

Layout: channels on partitions.  The host pre-transposes each core's shard
to [p, r, b, w] = x[r, b*128 + p, w]  (p: 128 partitions = c mod 128,
r: 228 rows, b: channel half, w: 196 = 14*14).  Every chunk DMA is then a
full 128-partition transfer (perfect SDMA engine balance, no striping) and
per-partition DRAM runs are contiguous.  Per chunk of rows:
  - gpsimd (SWDGE) DMA -> SBUF tile [128, nr*392] bf16
  - DVE segmented reduce_sum over w (bf16 in/out -> 2x packed mode)
    -> cs[p, 2r+b] channel sums
  - DVE square (bf16), TensorE ones-matmul reduces the 128 partitions
    -> psum[0, 2r+b] = sum_p cs^2
Tail: DVE combines b-pairs (psum -> norm^2[r]), ScalarE sqrt (scaled by
1/196), sync-DMA of [1, 228] f32 out.  The tiny 57x57 pattern-mixer math
runs on host in f64.
"""

import numpy as np
import ml_dtypes

import concourse.bacc as bacc
import concourse.mybir as mybir
import concourse.tile as tile
from concourse.bass_utils import run_bass_kernel_spmd

# Problem constants (hardcoded; kernel.py must be self-contained).
NUM_BASIC = 5
NUM_MIXED = 4
NUM_FRAME = 8
NUM_NODES = 7
NUM_SAMPLES = 8
M = 1 + NUM_NODES * NUM_FRAME  # 57

N_CORES = 8
B = 32
C = 256
HW = 196  # 14*14
ROWS_TOTAL = B * M          # 1824
R = ROWS_TOTAL // N_CORES   # 228 rows per core
P = 128                     # partitions (= C // 2)
CB = C // P                 # 2 channel halves per partition
SEG = CB * HW               # 392 elems per row per partition
FREE = R * SEG              # 89376 elems per partition

# Row chunking: big middle chunks, small tail chunks so the final
# reduce->matmul->sqrt chain after the last DMA is short.
CHUNK_ROWS = [24] * 9 + [8, 3, 1]
assert sum(CHUNK_ROWS) == R

LAST_RESULT = None
_NC_CACHE = None


def _build_nc(chunk_rows=None, bufs=4):
    if chunk_rows is None:
        chunk_rows = CHUNK_ROWS
    assert sum(chunk_rows) == R
    max_nr = max(chunk_rows)
    bf16 = mybir.dt.bfloat16
    f32 = mybir.dt.float32
    nc = bacc.Bacc(None)
    x = nc.declare_dram_parameter("x", [P, FREE], bf16, isOutput=False)
    out = nc.declare_dram_parameter("out", [1, R], f32, isOutput=True)
    with tile.TileContext(nc) as tc:
        with (
            tc.tile_pool(name="xt_pool", bufs=bufs) as xp,
            tc.tile_pool(name="singles", bufs=1) as sp,
            tc.tile_pool(name="psum", bufs=1, space="PSUM") as pp,
            nc.allow_low_precision(
                reason="bf16 channel-sums: norm output tolerates ~1e-3"
            ),
        ):
            ones = sp.tile([P, 1], bf16, tag="ones")
            nc.vector.memset(ones[:], 1.0)
            cs = sp.tile([P, CB * R], bf16, tag="cs")
            sq = sp.tile([P, CB * R], bf16, tag="sq")
            ps = pp.tile([1, CB * R], f32, tag="ps")
            nrm2 = sp.tile([1, R], f32, tag="nrm2")
            nrm = sp.tile([1, R], f32, tag="nrm")
            # Warm the ScalarE Sqrt table (1.3us load) during the DMA phase
            # so it isn't on the critical tail.
            warm = sp.tile([P, 1], f32, tag="warm")
            nc.scalar.activation(
                warm, ones, mybir.ActivationFunctionType.Sqrt, scale=1.0
            )
            r0 = 0
            for nr in chunk_rows:
                ns = CB * nr
                xt = xp.tile([P, max_nr * SEG], bf16, tag="xt")
                nc.gpsimd.dma_start(
                    out=xt[:, : nr * SEG],
                    in_=x[:, r0 * SEG : (r0 + nr) * SEG],
                )
                c0 = CB * r0
                nc.vector.reduce_sum(
                    cs[:, c0 : c0 + ns],
                    xt[:, : nr * SEG].rearrange("p (s w) -> p s w", w=HW),
                    axis=mybir.AxisListType.X,
                )
                nc.vector.tensor_mul(
                    sq[:, c0 : c0 + ns], cs[:, c0 : c0 + ns], cs[:, c0 : c0 + ns]
                )
                nc.tensor.matmul(
                    ps[:, c0 : c0 + ns],
                    ones[:],
                    sq[:, c0 : c0 + ns],
                    start=True,
                    stop=True,
                )
                r0 += nr
            nc.vector.reduce_sum(
                nrm2[:, :],
                ps[:, :].rearrange("p (r b) -> p r b", b=CB),
                axis=mybir.AxisListType.X,
            )
            nc.scalar.activation(
                nrm[:, :],
                nrm2[:, :],
                mybir.ActivationFunctionType.Sqrt,
                scale=1.0 / float(HW * HW),
            )
            nc.sync.dma_start(out=out[:, :], in_=nrm[:, :])
    nc.finalize()
    return nc


def _get_nc():
    global _NC_CACHE
    if _NC_CACHE is None:
        _NC_CACHE = _build_nc()
    return _NC_CACHE


def _zero_mask():
    mask = np.ones((M, M), dtype=np.float64)
    for i in range(NUM_SAMPLES):
        r = (1 + i) * NUM_NODES
        for c in range(1, M):
            if c % NUM_NODES != 0 and (c - 1) // NUM_NODES != i:
                mask[r, c] = 0.0
    return mask


def _pattern_mixer_np(mat, sigma, lin_w, lin_b, mixed_mat):
    mat = np.asarray(mat, np.float64)            # [5, 7, 7]
    sigma = np.asarray(sigma, np.float64)        # [4, 5, 1]
    lin_w = np.asarray(lin_w, np.float64)        # [4, 5]
    lin_b = np.asarray(lin_b, np.float64)        # [4]
    mixed_mat = np.asarray(mixed_mat, np.float64)  # [4, 57, 57]

    T2 = 2 * NUM_FRAME - 1  # 15
    dist = np.abs(np.arange(T2, dtype=np.float64) - (NUM_FRAME - 1))
    te = (1.0 / (np.sqrt(2.0 * np.pi) * sigma)) * np.exp(
        -(dist**2) / (2.0 * sigma**2)
    )  # [4, 5, 15]
    ce = 1.0 / (1.0 + np.exp(-te))
    mixed = (
        np.einsum("hbt,bnm,hb->hntm", ce, mat, lin_w)
        + lin_b[:, None, None, None]
    )
    mixed = np.maximum(mixed, 0.0).reshape(NUM_MIXED, NUM_NODES, T2 * NUM_NODES)
    blocks = [
        mixed[
            :,
            :,
            NUM_NODES * (NUM_SAMPLES - 1 - i) : NUM_NODES * (2 * NUM_SAMPLES - 1 - i),
        ]
        for i in range(NUM_SAMPLES)
    ]
    add_block = np.concatenate(blocks, axis=1)  # [4, 56, 56]
    mm = mixed_mat.copy()
    mm[:, 1:, 1:] += add_block
    mm *= _zero_mask()[None]
    deg = np.maximum(mm.sum(axis=2), 1.0) ** -0.5  # [4, 57]
    return (deg[:, :, None] * mm * deg[:, None, :]).astype(np.float32)


def kernel(mat, x, sigma, lin_w, lin_b, mixed_mat, alpha):
    global LAST_RESULT
    x = np.asarray(x, dtype=np.float32).reshape(ROWS_TOTAL, C, HW)
    xb = x.astype(ml_dtypes.bfloat16)  # device stream dtype
    in_maps = []
    for i in range(N_CORES):
        shard = xb[i * R : (i + 1) * R]  # [R, C, HW]
        # [r, (b p), w] -> [p, (r b w)]
        dev = np.ascontiguousarray(
            shard.reshape(R, CB, P, HW).transpose(2, 0, 1, 3)
        ).reshape(P, FREE)
        in_maps.append({"x": dev})
    nc = _get_nc()
    res = run_bass_kernel_spmd(nc, in_maps, core_ids=list(range(N_CORES)))
    LAST_RESULT = res
    norms = np.concatenate([r["out"][0] for r in res.results])  # [1824]
    nrm = norms.reshape(B, M)

    mp = _pattern_mixer_np(mat, sigma, lin_w, lin_b, mixed_mat)  # [4, 57, 57] f32
    alpha = np.asarray(alpha, np.float32).reshape(1, NUM_MIXED, 1, 1)
    out = mp[None] + alpha * nrm[:, None, None, :]
    return np.ascontiguousarray(out.astype(np.float32))
